# revision 20
# baseline (speedup 1.0000x reference)
"""Trainium2 Bass kernel for the ContextualAttentionLayer problem.

Math (per batch b):
    gene_proj = (genes * w_scalar) @ w_genes + b_genes            # [A]
    proj      = smiles[b] @ dense_kernel + dense_bias             # [T, A]
    x         = tanh(proj + gene_proj)                            # [T, A]
    xv        = x @ v                                             # [T]
    alphas    = softmax(xv)                                       # [T]
    out       = smiles[b].T @ alphas                              # [H]

Sharding: pure data parallel over batch, B=128 -> 16 batches per core on 8 cores.

Host prep: smiles is cast to bf16 and pre-transposed to [B, H, T] so every
device load is a plain contiguous DMA with H on partitions (the layout the
PE needs to contract over H). All small params are pre-blocked likewise.

On-device dataflow (per core, per batch):
  - stT [128h, T] x4 chunks loaded with one 1 MiB DMA.
  - projT[a, t] accumulated on PE over the 4 h-chunks (bf16 -> fp32 PSUM).
  - ACT tanh with per-partition bias (gene_proj + dense_bias in [A, batch]
    layout, computed on device in a small fp32 prologue) -> x.T bf16.
  - xv row: PE matmul v.T @ x.T -> PSUM [1, T].
  - ACT exp (no max subtraction: |xv| <= sum|v| ~ 20, safe in fp32) with
    accum_out -> p row fp32 + S = sum(p).
  - DVE casts p -> bf16, gpsimd broadcasts it to 128 partitions.
  - step4: (stT_chunk * p_bcast) with accum_out along T gives the
    unnormalized out[h] per 128-wide h chunk (3 chunks on DVE, 1 on gpsimd).
  - Host divides p and out_col by S (normalization) and restores layouts.
"""

import numpy as np
import ml_dtypes
from contextlib import ExitStack

B, T, H, A, G = 128, 1024, 512, 256, 2128
GC = 17           # ceil(G/128) g-chunks, padded
GP = GC * 128     # 2176
NCORES = 8
BS = B // NCORES  # 16 batches per core
HC = H // 128     # 4 h-chunks
AC = A // 128     # 2 a-chunks

_CACHE = {}


def _build_program():
    import concourse.bacc as bacc
    import concourse.tile as tile
    import concourse.mybir as mybir

    dt = mybir.dt
    f32, bf16 = dt.float32, dt.bfloat16
    Tanh = mybir.ActivationFunctionType.Tanh
    Exp = mybir.ActivationFunctionType.Exp
    Ident = mybir.ActivationFunctionType.Identity
    mult = mybir.AluOpType.mult
    add = mybir.AluOpType.add

    nc = bacc.Bacc("TRN2", target_bir_lowering=False, debug=False)

    smilesT_d = nc.dram_tensor("smilesT_bf", [BS, H, T], bf16, kind="ExternalInput").ap()
    wden_d = nc.dram_tensor("wden_bf", [128, HC, A], bf16, kind="ExternalInput").ap()
    vcols_d = nc.dram_tensor("v_cols", [128, AC], bf16, kind="ExternalInput").ap()
    genesT_d = nc.dram_tensor("genesT", [128, GC, BS], bf16, kind="ExternalInput").ap()
    wg_d = nc.dram_tensor("wg", [128, GC, A], bf16, kind="ExternalInput").ap()
    biasT_d = nc.dram_tensor("biasT", [128, AC], f32, kind="ExternalInput").ap()
    p_o = nc.dram_tensor("p_out", [BS, T], f32, kind="ExternalOutput").ap()
    s_o = nc.dram_tensor("s_out", [1, BS], f32, kind="ExternalOutput").ap()
    out_o = nc.dram_tensor("out_col", [BS, 128, HC], f32, kind="ExternalOutput").ap()

    with tile.TileContext(nc) as tc, ExitStack() as ctx:
        const_pool = ctx.enter_context(tc.tile_pool(name="const", bufs=1))
        wden = const_pool.tile([128, HC, A], bf16)
        nc.sync.dma_start(wden[:], wden_d)
        vcols = const_pool.tile([128, AC], bf16)
        nc.sync.dma_start(vcols[:], vcols_d)
        biasT = const_pool.tile([128, AC], f32)
        nc.sync.dma_start(biasT[:], biasT_d)
        gpT = const_pool.tile([128, AC, BS], f32)   # gene_proj + dense_bias, [a, b]
        S_all = const_pool.tile([1, BS], f32)       # per-batch softmax denominators

        # ---- prologue: gene_projT[a, b] = sum_g wg[g, a] * genesT[g, b] ----
        with ExitStack() as pctx:
            ppool = pctx.enter_context(tc.tile_pool(name="prolog", bufs=1))
            ppsum = pctx.enter_context(tc.tile_pool(name="prolog_ps", bufs=2, space="PSUM"))
            wg_s = ppool.tile([128, GC, A], bf16)
            nc.sync.dma_start(wg_s[:], wg_d)
            gT_s = ppool.tile([128, GC, BS], bf16)
            nc.sync.dma_start(gT_s[:], genesT_d)
            for ah in range(AC):
                pg = ppsum.tile([128, BS], f32)
                for c in range(GC):
                    nc.tensor.matmul(
                        pg[:],
                        wg_s[:, c, ah * 128:(ah + 1) * 128],
                        gT_s[:, c, :],
                        start=(c == 0), stop=(c == GC - 1),
                    )
                nc.scalar.activation(gpT[:, ah, :], pg[:], Ident, bias=biasT[:, ah:ah + 1])

        # ---- main loop over batches ----
        st_pool = ctx.enter_context(tc.tile_pool(name="stT", bufs=6))
        x_pool = ctx.enter_context(tc.tile_pool(name="xT", bufs=6))
        bcast_pool = ctx.enter_context(tc.tile_pool(name="bcast", bufs=3))
        scr_pool = ctx.enter_context(tc.tile_pool(name="scr", bufs=4))
        row_pool = ctx.enter_context(tc.tile_pool(name="rows", bufs=8))
        ocol_pool = ctx.enter_context(tc.tile_pool(name="ocol", bufs=4))
        pj_psum = ctx.enter_context(tc.tile_pool(name="pj_ps", bufs=3, space="PSUM"))
        xv_psum = ctx.enter_context(tc.tile_pool(name="xv_ps", bufs=1, space="PSUM"))

        # Software-pipelined emission: batch b's xv/softmax/step4 are emitted
        # after batch b+1's proj matmuls, so the PE never stalls on tanh at
        # batch boundaries (keeps HAM un-throttled).
        state = {}  # b -> (stT, xTs)

        def emit_front(b):
            stT = st_pool.tile([128, HC, T], bf16)
            nc.sync.dma_start(
                stT[:], smilesT_d[b].rearrange("(c p) t -> p c t", p=128))
            xTs = []
            for ah in range(AC):
                pj = pj_psum.tile([128, T], f32, tag="pj")
                for th in range(2):
                    for c in range(HC):
                        nc.tensor.matmul(
                            pj[:, th * 512:(th + 1) * 512],
                            wden[:, c, ah * 128:(ah + 1) * 128],
                            stT[:, c, th * 512:(th + 1) * 512],
                            start=(c == 0), stop=(c == HC - 1),
                        )
                xT = x_pool.tile([128, T], bf16, tag="xT")
                nc.scalar.activation(xT[:], pj[:], Tanh, bias=gpT[:, ah, b:b + 1])
                xTs.append(xT)
            state[b] = (stT, xTs)

        def emit_back(b):
            stT, xTs = state.pop(b)
            xv = xv_psum.tile([1, T], f32, tag="xv")
            for th in range(2):
                for ah in range(AC):
                    nc.tensor.matmul(
                        xv[:, th * 512:(th + 1) * 512],
                        vcols[:, ah:ah + 1],
                        xTs[ah][:, th * 512:(th + 1) * 512],
                        start=(ah == 0), stop=(ah == AC - 1),
                    )
            p_row = row_pool.tile([1, T], f32, tag="p_row")
            nc.scalar.activation(p_row[:], xv[:], Exp, accum_out=S_all[:, b:b + 1])
            # stage p in DRAM (doubles as the p_out output), then read it back
            # replicated to all 128 partitions with a cast to bf16 (SWDGE).
            nc.gpsimd.dma_start(p_o[b:b + 1, :], p_row[:])
            ab = bcast_pool.tile([128, T], bf16)
            nc.gpsimd.dma_start(ab[:], p_o[b:b + 1, :].broadcast_to([128, T]))
            oc = ocol_pool.tile([128, HC], f32)
            for c in range(HC):
                scr = scr_pool.tile([128, T], bf16, tag="scr")
                nc.vector.scalar_tensor_tensor(
                    scr[:], stT[:, c, :], 1.0, ab[:],
                    mult, mult, accum_out=oc[:, c:c + 1])
            nc.gpsimd.dma_start(out_o[b], oc[:])

        for b in range(BS):
            emit_front(b)
            if b >= 1:
                emit_back(b - 1)
        emit_back(BS - 1)

        nc.gpsimd.dma_start(s_o[:], S_all[:])

    nc.compile()
    return nc


def _prep_host(genes, smiles, w_num_gene_features, w_genes, b_genes,
               dense_kernel, dense_bias, v):
    bf16 = ml_dtypes.bfloat16
    f32 = np.float32

    # smiles: [B, T, H] -> transposed [B, H, T] bf16
    smilesT = np.ascontiguousarray(
        smiles.astype(bf16).transpose(0, 2, 1))

    # genesT scaled by w_num_gene_features, padded: [128, GC, B]
    gsc = (genes.astype(f32) * f32(w_num_gene_features[0]))  # [B, G]
    gpad = np.zeros((B, GP), dtype=f32)
    gpad[:, :G] = gsc
    genesT = np.ascontiguousarray(
        gpad.reshape(B, GC, 128).transpose(2, 1, 0)).astype(bf16)  # [128, GC, B]

    wgpad = np.zeros((GP, A), dtype=f32)
    wgpad[:G] = w_genes.astype(f32)
    wg_h = np.ascontiguousarray(
        wgpad.reshape(GC, 128, A).transpose(1, 0, 2)).astype(bf16)  # [128, GC, A]

    wden_h = np.ascontiguousarray(
        dense_kernel.astype(f32).reshape(HC, 128, A).transpose(1, 0, 2)
    ).astype(bf16)  # [128, HC, A]

    vcols_h = np.ascontiguousarray(v.astype(f32).reshape(AC, 128).T).astype(bf16)  # [128, AC]
    biasT_h = np.ascontiguousarray(
        (b_genes.astype(f32) + dense_bias.astype(f32)).reshape(AC, 128).T
    )  # [128, AC]

    return smilesT, genesT, wg_h, wden_h, vcols_h, biasT_h


def _make_in_maps(inp):
    smilesT, genesT, wg_h, wden_h, vcols_h, biasT_h = _prep_host(
        np.asarray(inp["genes"]), np.asarray(inp["smiles"]),
        np.asarray(inp["w_num_gene_features"]), np.asarray(inp["w_genes"]),
        np.asarray(inp["b_genes"]), np.asarray(inp["dense_kernel"]),
        np.asarray(inp["dense_bias"]), np.asarray(inp["v"]))
    in_maps = []
    for i in range(NCORES):
        sl = slice(i * BS, (i + 1) * BS)
        in_maps.append({
            "smilesT_bf": np.ascontiguousarray(smilesT[sl]),
            "wden_bf": wden_h,
            "v_cols": vcols_h,
            "genesT": np.ascontiguousarray(genesT[:, :, sl]),
            "wg": wg_h,
            "biasT": biasT_h,
        })
    return in_maps


def _assemble(results):
    outs, alphas = [], []
    for r in results:
        S = r["s_out"].reshape(BS, 1)                      # [BS, 1]
        alphas.append(r["p_out"] / S)                      # [BS, T]
        oc = r["out_col"]                                  # [BS, 128, HC]
        outs.append(oc.transpose(0, 2, 1).reshape(BS, H) / S)
    return (np.concatenate(outs, axis=0).astype(np.float32),
            np.concatenate(alphas, axis=0).astype(np.float32))


def kernel(genes, smiles, w_num_gene_features, w_genes, b_genes,
           dense_kernel, dense_bias, v):
    from concourse.bass_utils import run_bass_kernel_spmd

    if "nc" not in _CACHE:
        _CACHE["nc"] = _build_program()
    nc = _CACHE["nc"]

    in_maps = _make_in_maps(dict(
        genes=genes, smiles=smiles, w_num_gene_features=w_num_gene_features,
        w_genes=w_genes, b_genes=b_genes, dense_kernel=dense_kernel,
        dense_bias=dense_bias, v=v))

    res = run_bass_kernel_spmd(nc, in_maps, core_ids=list(range(NCORES)))
    _CACHE["last_result"] = res
    return _assemble(res.results)


# revision 21
# speedup vs baseline: 1.0887x; 1.0887x over previous
"""Trainium2 Bass kernel for the ContextualAttentionLayer problem.

Math (per batch b):
    gene_proj = (genes * w_scalar) @ w_genes + b_genes            # [A]
    proj      = smiles[b] @ dense_kernel + dense_bias             # [T, A]
    x         = tanh(proj + gene_proj)                            # [T, A]
    xv        = x @ v                                             # [T]
    alphas    = softmax(xv)                                       # [T]
    out       = smiles[b].T @ alphas                              # [H]

Sharding: pure data parallel over batch, B=128 -> 16 batches per core on 8 cores.

Host prep: smiles is cast to bf16 and pre-transposed to [B, H, T] so every
device load is a plain contiguous DMA with H on partitions (the layout the
PE needs to contract over H). All small params are pre-blocked likewise.

On-device dataflow (per core, per batch):
  - stT [128h, T] x4 chunks loaded with one 1 MiB DMA.
  - projT[a, t] accumulated on PE over the 4 h-chunks (bf16 -> fp32 PSUM).
  - ACT tanh with per-partition bias (gene_proj + dense_bias in [A, batch]
    layout, computed on device in a small fp32 prologue) -> x.T bf16.
  - xv row: PE matmul v.T @ x.T -> PSUM [1, T].
  - ACT exp (no max subtraction: |xv| <= sum|v| ~ 20, safe in fp32) with
    accum_out -> p row fp32 + S = sum(p).
  - DVE casts p -> bf16, gpsimd broadcasts it to 128 partitions.
  - step4: (stT_chunk * p_bcast) with accum_out along T gives the
    unnormalized out[h] per 128-wide h chunk (3 chunks on DVE, 1 on gpsimd).
  - Host divides p and out_col by S (normalization) and restores layouts.
"""

import numpy as np
import ml_dtypes
from contextlib import ExitStack

B, T, H, A, G = 128, 1024, 512, 256, 2128
GC = 17           # ceil(G/128) g-chunks, padded
GP = GC * 128     # 2176
NCORES = 8
BS = B // NCORES  # 16 batches per core
HC = H // 128     # 4 h-chunks
AC = A // 128     # 2 a-chunks

_CACHE = {}


def _build_program():
    import concourse.bacc as bacc
    import concourse.tile as tile
    import concourse.mybir as mybir

    dt = mybir.dt
    f32, bf16 = dt.float32, dt.bfloat16
    Tanh = mybir.ActivationFunctionType.Tanh
    Exp = mybir.ActivationFunctionType.Exp
    Ident = mybir.ActivationFunctionType.Identity
    mult = mybir.AluOpType.mult
    add = mybir.AluOpType.add

    nc = bacc.Bacc("TRN2", target_bir_lowering=False, debug=False)

    smilesT_d = nc.dram_tensor("smilesT_bf", [BS, H, T], bf16, kind="ExternalInput").ap()
    wden_d = nc.dram_tensor("wden_bf", [128, HC, A], bf16, kind="ExternalInput").ap()
    vcols_d = nc.dram_tensor("v_cols", [128, AC], bf16, kind="ExternalInput").ap()
    genesT_d = nc.dram_tensor("genesT", [128, GC, BS], bf16, kind="ExternalInput").ap()
    wg_d = nc.dram_tensor("wg", [128, GC, A], bf16, kind="ExternalInput").ap()
    biasT_d = nc.dram_tensor("biasT", [128, AC], f32, kind="ExternalInput").ap()
    p_o = nc.dram_tensor("p_out", [BS, T], f32, kind="ExternalOutput").ap()
    s_o = nc.dram_tensor("s_out", [1, BS], f32, kind="ExternalOutput").ap()
    out_o = nc.dram_tensor("out_col", [BS, 128, HC], f32, kind="ExternalOutput").ap()

    with tile.TileContext(nc) as tc, ExitStack() as ctx:
        const_pool = ctx.enter_context(tc.tile_pool(name="const", bufs=1))
        wden = const_pool.tile([128, HC, A], bf16)
        nc.sync.dma_start(wden[:], wden_d)
        vcols = const_pool.tile([128, AC], bf16)
        nc.sync.dma_start(vcols[:], vcols_d)
        biasT = const_pool.tile([128, AC], f32)
        nc.sync.dma_start(biasT[:], biasT_d)
        gpT = const_pool.tile([128, AC, BS], f32)   # gene_proj + dense_bias, [a, b]
        S_all = const_pool.tile([1, BS], f32)       # per-batch softmax denominators

        # ---- prologue: gene_projT[a, b] = sum_g wg[g, a] * genesT[g, b] ----
        # Tiles live in the persistent pool: releasing them would let the
        # batch-loop pools reuse their SBUF, adding a WAR dependency that
        # serializes the first smiles loads behind the whole prologue.
        with ExitStack() as pctx:
            ppsum = pctx.enter_context(tc.tile_pool(name="prolog_ps", bufs=2, space="PSUM"))
            wg_s = const_pool.tile([128, GC, A], bf16)
            nc.sync.dma_start(wg_s[:], wg_d)
            gT_s = const_pool.tile([128, GC, BS], bf16)
            nc.sync.dma_start(gT_s[:], genesT_d)
            for ah in range(AC):
                pg = ppsum.tile([128, BS], f32)
                for c in range(GC):
                    nc.tensor.matmul(
                        pg[:],
                        wg_s[:, c, ah * 128:(ah + 1) * 128],
                        gT_s[:, c, :],
                        start=(c == 0), stop=(c == GC - 1),
                    )
                nc.scalar.activation(gpT[:, ah, :], pg[:], Ident, bias=biasT[:, ah:ah + 1])

        # ---- main loop over batches ----
        st_pool = ctx.enter_context(tc.tile_pool(name="stT", bufs=6))
        x_pool = ctx.enter_context(tc.tile_pool(name="xT", bufs=6))
        bcast_pool = ctx.enter_context(tc.tile_pool(name="bcast", bufs=3))
        scr_pool = ctx.enter_context(tc.tile_pool(name="scr", bufs=4))
        row_pool = ctx.enter_context(tc.tile_pool(name="rows", bufs=8))
        ocol_pool = ctx.enter_context(tc.tile_pool(name="ocol", bufs=4))
        pj_psum = ctx.enter_context(tc.tile_pool(name="pj_ps", bufs=3, space="PSUM"))
        xv_psum = ctx.enter_context(tc.tile_pool(name="xv_ps", bufs=1, space="PSUM"))

        # Software-pipelined emission: batch b's xv/softmax/step4 are emitted
        # after batch b+1's proj matmuls, so the PE never stalls on tanh at
        # batch boundaries (keeps HAM un-throttled).
        state = {}  # b -> (stT, xTs)

        def emit_front(b):
            stT = st_pool.tile([128, HC, T], bf16)
            nc.sync.dma_start(
                stT[:], smilesT_d[b].rearrange("(c p) t -> p c t", p=128))
            xTs = []
            for ah in range(AC):
                pj = pj_psum.tile([128, T], f32, tag="pj")
                for th in range(2):
                    for c in range(HC):
                        nc.tensor.matmul(
                            pj[:, th * 512:(th + 1) * 512],
                            wden[:, c, ah * 128:(ah + 1) * 128],
                            stT[:, c, th * 512:(th + 1) * 512],
                            start=(c == 0), stop=(c == HC - 1),
                        )
                xT = x_pool.tile([128, T], bf16, tag="xT")
                nc.scalar.activation(xT[:], pj[:], Tanh, bias=gpT[:, ah, b:b + 1])
                xTs.append(xT)
            state[b] = (stT, xTs)

        def emit_back(b):
            stT, xTs = state.pop(b)
            xv = xv_psum.tile([1, T], f32, tag="xv")
            for th in range(2):
                for ah in range(AC):
                    nc.tensor.matmul(
                        xv[:, th * 512:(th + 1) * 512],
                        vcols[:, ah:ah + 1],
                        xTs[ah][:, th * 512:(th + 1) * 512],
                        start=(ah == 0), stop=(ah == AC - 1),
                    )
            p_row = row_pool.tile([1, T], f32, tag="p_row")
            nc.scalar.activation(p_row[:], xv[:], Exp, accum_out=S_all[:, b:b + 1])
            # stage p in DRAM (doubles as the p_out output), then read it back
            # replicated to all 128 partitions with a cast to bf16 (SWDGE).
            nc.gpsimd.dma_start(p_o[b:b + 1, :], p_row[:])
            ab = bcast_pool.tile([128, T], bf16)
            nc.gpsimd.dma_start(ab[:], p_o[b:b + 1, :].broadcast_to([128, T]))
            oc = ocol_pool.tile([128, HC], f32)
            for c in range(HC):
                scr = scr_pool.tile([128, T], bf16, tag="scr")
                nc.vector.scalar_tensor_tensor(
                    scr[:], stT[:, c, :], 1.0, ab[:],
                    mult, mult, accum_out=oc[:, c:c + 1])
            nc.gpsimd.dma_start(out_o[b], oc[:])

        for b in range(BS):
            emit_front(b)
            if b >= 1:
                emit_back(b - 1)
        emit_back(BS - 1)

        nc.gpsimd.dma_start(s_o[:], S_all[:])

    nc.compile()
    return nc


def _prep_host(genes, smiles, w_num_gene_features, w_genes, b_genes,
               dense_kernel, dense_bias, v):
    bf16 = ml_dtypes.bfloat16
    f32 = np.float32

    # smiles: [B, T, H] -> transposed [B, H, T] bf16
    smilesT = np.ascontiguousarray(
        smiles.astype(bf16).transpose(0, 2, 1))

    # genesT scaled by w_num_gene_features, padded: [128, GC, B]
    gsc = (genes.astype(f32) * f32(w_num_gene_features[0]))  # [B, G]
    gpad = np.zeros((B, GP), dtype=f32)
    gpad[:, :G] = gsc
    genesT = np.ascontiguousarray(
        gpad.reshape(B, GC, 128).transpose(2, 1, 0)).astype(bf16)  # [128, GC, B]

    wgpad = np.zeros((GP, A), dtype=f32)
    wgpad[:G] = w_genes.astype(f32)
    wg_h = np.ascontiguousarray(
        wgpad.reshape(GC, 128, A).transpose(1, 0, 2)).astype(bf16)  # [128, GC, A]

    wden_h = np.ascontiguousarray(
        dense_kernel.astype(f32).reshape(HC, 128, A).transpose(1, 0, 2)
    ).astype(bf16)  # [128, HC, A]

    vcols_h = np.ascontiguousarray(v.astype(f32).reshape(AC, 128).T).astype(bf16)  # [128, AC]
    biasT_h = np.ascontiguousarray(
        (b_genes.astype(f32) + dense_bias.astype(f32)).reshape(AC, 128).T
    )  # [128, AC]

    return smilesT, genesT, wg_h, wden_h, vcols_h, biasT_h


def _make_in_maps(inp):
    smilesT, genesT, wg_h, wden_h, vcols_h, biasT_h = _prep_host(
        np.asarray(inp["genes"]), np.asarray(inp["smiles"]),
        np.asarray(inp["w_num_gene_features"]), np.asarray(inp["w_genes"]),
        np.asarray(inp["b_genes"]), np.asarray(inp["dense_kernel"]),
        np.asarray(inp["dense_bias"]), np.asarray(inp["v"]))
    in_maps = []
    for i in range(NCORES):
        sl = slice(i * BS, (i + 1) * BS)
        in_maps.append({
            "smilesT_bf": np.ascontiguousarray(smilesT[sl]),
            "wden_bf": wden_h,
            "v_cols": vcols_h,
            "genesT": np.ascontiguousarray(genesT[:, :, sl]),
            "wg": wg_h,
            "biasT": biasT_h,
        })
    return in_maps


def _assemble(results):
    outs, alphas = [], []
    for r in results:
        S = r["s_out"].reshape(BS, 1)                      # [BS, 1]
        alphas.append(r["p_out"] / S)                      # [BS, T]
        oc = r["out_col"]                                  # [BS, 128, HC]
        outs.append(oc.transpose(0, 2, 1).reshape(BS, H) / S)
    return (np.concatenate(outs, axis=0).astype(np.float32),
            np.concatenate(alphas, axis=0).astype(np.float32))


def kernel(genes, smiles, w_num_gene_features, w_genes, b_genes,
           dense_kernel, dense_bias, v):
    from concourse.bass_utils import run_bass_kernel_spmd

    if "nc" not in _CACHE:
        _CACHE["nc"] = _build_program()
    nc = _CACHE["nc"]

    in_maps = _make_in_maps(dict(
        genes=genes, smiles=smiles, w_num_gene_features=w_num_gene_features,
        w_genes=w_genes, b_genes=b_genes, dense_kernel=dense_kernel,
        dense_bias=dense_bias, v=v))

    res = run_bass_kernel_spmd(nc, in_maps, core_ids=list(range(NCORES)))
    _CACHE["last_result"] = res
    return _assemble(res.results)
